# revision 49
# baseline (speedup 1.0000x reference)
"""Trainium2 Bass kernel for AffinityNet (2-layer GCN + mean-pool + MLP head).

v2 strategy (8 NeuronCores, SPMD):
  - Node slots: 8 cores x 13 superchunks x 512 slots (npad=53248), balanced
    in-degree bin packing.
  - Layer 1: dst-partitioned. Per (sc, half): dma_gather x rows (fp16, table
    split in 2 halves for int16 idx), one-hot S on DVE (fp16 iota), PE
    accumulates agg^T[f, slot] in PSUM. Self-loops via diagonal-S matmuls on
    the SBUF-resident x block (no gather). Then W1 matmul + ReLU; node-major
    h_n kept in SBUF; h1 block written to local DRAM.
  - h1 exchange: ONE pair AllGather ([[0,1],[2,3],..]) -> h1_pair [2*block]
    (pair cores share HBM, so this is cheap on HW).
  - Layer 2: src-PAIR-partitioned, target-parity split: core c handles edges
    with src in its pair and dst core ≡ c (mod 2). Gathers read h1_pair (one
    table, no halves). Partial aggregations agg^T in fp16 -> partial[tc_slot,
    sc] DRAM, 3 sc-chunks; dual 4-core ReduceScatter ([[0,2,4,6],[1,3,5,7]])
    delivers summed agg2 for own slots. Self-loop + bias + W2 + ReLU + pool
    matmul epilogue per sc; pool AllReduce (64KB) + replicated MLP head.
"""

import sys

sys.path.insert(0, "/opt/trn_rl_repo")

import math

import numpy as np

from concourse import bacc, mybir, tile
from concourse.bass_types import AP
from concourse.bass_utils import run_bass_kernel_spmd
from concourse.masks import make_identity

F32 = mybir.dt.float32
F16 = mybir.dt.float16
I16 = mybir.dt.int16
P = 128
NC = 8
NUM_GRAPHS = 128
BN_EPS = 1e-5
import os

GCALL = int(os.environ.get("K_GCALL", "8"))  # tiles per dma_gather call (1024 idx HW cap)
AGCH = 13 if int(os.environ.get("K_MANUALX", "0")) else int(os.environ.get("K_AGCH", "13"))  # AG chunk size (sc)
SFRAC = int(os.environ.get("K_SFRAC", "0"))   # 1/SFRAC S-builds on gpsimd
RSDELAY = int(os.environ.get("K_RSDELAY", "1"))  # delay RS issue by one chunk
MANUALX = int(os.environ.get("K_MANUALX", "0"))  # manual pair h1 exchange
CCENG = os.environ.get("K_CCENG", "pool")  # engine queue for collectives


class Cfg:
    def __init__(self, n, nsc, t1, t2):
        self.n = n
        self.nc = NC
        self.nsc = nsc
        self.t1 = t1  # [nsc, 2, 4] tiles per (sc, half, sub), shared
        self.t2 = t2  # [nsc, 4, 4] tiles per (sc, j, sub), shared
        self.bins_per_core = nsc * 4
        self.block = self.bins_per_core * P
        self.npad = NC * self.block
        self.half = self.npad // 2
        # L1 layout: segments (sc, half); within: sub-major tiles
        self.T1 = t1.sum(axis=2)          # [nsc, 2]
        self.off1 = np.concatenate([[0], np.cumsum(self.T1.reshape(-1))])[
            :-1].reshape(self.T1.shape)   # [nsc, 2] global tile offset
        self.T1_tot = int(self.T1.sum())
        self.toff1 = np.cumsum(t1, axis=2) - t1  # [nsc,2,4] within-seg offset
        # L2 layout: segments (sc); within: (j, sub)-major tiles
        self.T2 = t2.sum(axis=(1, 2))     # [nsc]
        self.off2 = np.concatenate([[0], np.cumsum(self.T2)])[:-1]
        self.T2_tot = int(self.T2.sum())
        t2f = t2.reshape(self.nsc, 16)
        self.toff2 = (np.cumsum(t2f, axis=1) - t2f).reshape(t2.shape)
        # RS sc-chunks
        import os as _os
        _l = _os.environ.get("K_RSCHL", "6,5,2" if nsc == 13 else "")
        if _l:
            ch = [int(x) for x in _l.replace(";", ",").split(",")]
            assert sum(ch) == nsc
        else:
            nch = int(_os.environ.get("K_RSCH", "3"))
            ch = [nsc // nch + (1 if i < nsc % nch else 0) for i in range(nch)]
        self.chunks = []
        s = 0
        for c in ch:
            self.chunks.append((s, s + c))
            s += c
        # pair-AllGather sc-chunks
        self.agchunks = [(a, min(a + AGCH, nsc)) for a in range(0, nsc, AGCH)]

    def __repr__(self):
        return (f"Cfg(n={self.n}, nsc={self.nsc}, T1_tot={self.T1_tot}, "
                f"T2_tot={self.T2_tot}, chunks={self.chunks})")


def _pack_bins(deg, nbins, cap):
    import heapq

    n = len(deg)
    order = np.argsort(-deg, kind="stable")
    heap = [(0.0, b) for b in range(nbins)]
    heapq.heapify(heap)
    fill = np.zeros(nbins, np.int64)
    rowof = np.empty(n, np.int64)
    for node in order:
        while True:
            load, b = heapq.heappop(heap)
            if fill[b] < cap:
                break
        rowof[node] = b * cap + fill[b]
        fill[b] += 1
        if fill[b] < cap:
            heapq.heappush(heap, (load + float(deg[node]), b))
    return rowof


def _wrap_seg(flat):
    """[nc, S] int16 (S % 128 == 0) -> [nc, 128, S//16] gather idx layout."""
    nc_, s = flat.shape
    a = flat.reshape(nc_, s // 16, 16)
    a = np.swapaxes(a, -1, -2)  # [nc, 16, S//16]
    return np.tile(a, (1, 8, 1)).astype(np.int16)


def _pos_in_group(key, ngroups):
    cnts = np.bincount(key, minlength=ngroups)
    order = np.argsort(key, kind="stable")
    starts = np.zeros(ngroups, np.int64)
    starts[1:] = np.cumsum(cnts)[:-1]
    pos = np.empty(len(key), np.int64)
    pos[order] = np.arange(len(key)) - starts[key[order]]
    return pos, cnts


def _prep(x, edge_index, batch, nc_cores=8):
    assert nc_cores == NC
    x = np.ascontiguousarray(np.asarray(x, np.float32))
    edge_index = np.asarray(edge_index)
    batch = np.asarray(batch).astype(np.int64)
    n, f = x.shape
    assert f == P

    src = edge_index[0].astype(np.int64)
    dst = edge_index[1].astype(np.int64)

    deg = np.bincount(dst, minlength=n).astype(np.float64) + 1.0
    dinv = (1.0 / np.sqrt(deg)).astype(np.float32)
    snorm_node = (dinv * dinv).astype(np.float32)

    nbins_needed = math.ceil(n / P)
    bins_per_core = math.ceil(nbins_needed / (NC * 4)) * 4
    nsc = bins_per_core // 4
    nbins = NC * bins_per_core
    rowof = _pack_bins(deg, nbins, P)
    block = bins_per_core * P
    npad = nbins * P
    half = npad // 2
    assert half <= 32767 and 2 * block <= 32768

    drow = rowof[dst]
    srow = rowof[src]
    norm_e = (dinv[src] * dinv[dst]).astype(np.float32)

    core1 = drow // block
    sc_d = (drow % block) // 512
    sub_d = (drow % 512) // P
    slot_d = (drow % P).astype(np.float32)

    # ---- L1: dst-partitioned, src idx into x table halves ----
    halfe = (srow >= half).astype(np.int64)
    idx1v = (srow - halfe * half).astype(np.int64)
    key1 = ((core1 * nsc + sc_d) * 2 + halfe) * 4 + sub_d
    pos1, cnts1 = _pos_in_group(key1, NC * nsc * 2 * 4)
    c1 = cnts1.reshape(NC, nsc, 2, 4)
    t1 = np.ceil(c1.max(axis=0) / P).astype(np.int64)  # [nsc,2,4]
    t1 = np.maximum(t1, 1)

    # ---- L2: src-pair partitioned, dst parity split ----
    pair_s = srow // (2 * block)
    # h1_pair interleaved layout: per AG chunk of AGCH superchunks,
    # [member, sc-in-chunk, 512] blocks (member-major within the chunk)
    scs_s = (srow % block) // 512
    member_s = (srow // block) % 2
    scp_s = scs_s // AGCH
    chunk_nsc = np.minimum(AGCH, nsc - scp_s * AGCH)
    lrow2 = (scp_s * AGCH * 1024 + member_s * chunk_nsc * 512
             + (scs_s % AGCH) * 512 + (srow % 512))
    parity = core1 % 2
    j2 = core1 // 2
    pcore = pair_s * 2 + parity
    key2 = ((pcore * nsc + sc_d) * 4 + j2) * 4 + sub_d
    pos2, cnts2 = _pos_in_group(key2, NC * nsc * 4 * 4)
    c2 = cnts2.reshape(NC, nsc, 4, 4)
    t2 = np.ceil(c2.max(axis=0) / P).astype(np.int64)  # [nsc, 4(j), 4(sub)]
    t2 = np.maximum(t2, 1)

    cfg = Cfg(n=n, nsc=nsc, t1=t1, t2=t2)

    # ---- L1 arrays ----
    s1 = (cfg.off1[sc_d, halfe] + cfg.toff1[sc_d, halfe, sub_d]) * P + pos1
    idx1_f = np.zeros((NC, cfg.T1_tot * P), np.int64)
    slot1_f = np.zeros((NC, cfg.T1_tot * P), np.float32)
    norm1_f = np.zeros((NC, cfg.T1_tot * P), np.float32)
    idx1_f[core1, s1] = idx1v
    slot1_f[core1, s1] = slot_d
    norm1_f[core1, s1] = norm_e
    idx1_w = _wrap_seg(idx1_f.astype(np.int16))
    slot1_w = np.swapaxes(slot1_f.reshape(NC, cfg.T1_tot, P), -1, -2).copy()
    norm1_w = np.swapaxes(norm1_f.reshape(NC, cfg.T1_tot, P), -1, -2).copy()

    # ---- L2 arrays ----
    s2 = (cfg.off2[sc_d] + cfg.toff2[sc_d, j2, sub_d]) * P + pos2
    idx2_f = np.zeros((NC, cfg.T2_tot * P), np.int64)
    slot2_f = np.zeros((NC, cfg.T2_tot * P), np.float32)
    norm2_f = np.zeros((NC, cfg.T2_tot * P), np.float32)
    idx2_f[pcore, s2] = lrow2
    slot2_f[pcore, s2] = slot_d
    norm2_f[pcore, s2] = norm_e
    idx2_w = _wrap_seg(idx2_f.astype(np.int16))
    slot2_w = np.swapaxes(slot2_f.reshape(NC, cfg.T2_tot, P), -1, -2).copy()
    norm2_w = np.swapaxes(norm2_f.reshape(NC, cfg.T2_tot, P), -1, -2).copy()

    # ---- node-slot statics ----
    x_perm = np.zeros((npad, P), np.float16)
    x_perm[rowof] = x.astype(np.float16)
    xblock_w = x_perm.reshape(NC, bins_per_core, P, P).transpose(0, 2, 1, 3).copy()
    snorm_full = np.zeros(npad, np.float32)
    snorm_full[rowof] = snorm_node
    snorm_w = snorm_full.reshape(NC, bins_per_core, P).transpose(0, 2, 1).copy()

    cnt_g = np.bincount(batch, minlength=NUM_GRAPHS).astype(np.float32)
    invc = 1.0 / np.maximum(cnt_g, 1.0)
    gid_full = -np.ones(npad, np.float32)
    inv_full = np.zeros(npad, np.float32)
    gid_full[rowof] = batch.astype(np.float32)
    inv_full[rowof] = invc[batch]
    gid_w = gid_full.reshape(NC, bins_per_core, P).transpose(0, 2, 1).copy()
    inv_w = inv_full.reshape(NC, bins_per_core, P).transpose(0, 2, 1).copy()

    shared = {
        "x_lo": np.ascontiguousarray(x_perm[:half]),
        "x_hi": np.ascontiguousarray(x_perm[half:]),
        "iota16": np.tile(np.arange(P, dtype=np.float16), (P, 1)),
        "sid": np.arange(P, dtype=np.float32).reshape(P, 1),
    }
    percore = {
        "idx1": idx1_w, "slot1": slot1_w, "norm1": norm1_w,
        "idx2": idx2_w, "slot2": slot2_w, "norm2": norm2_w,
        "xblock": xblock_w, "snorm": snorm_w, "gid": gid_w, "inv": inv_w,
    }
    return cfg, shared, percore


def _build(cfg, ablate=(), reps=1):
    """Build the SPMD Bass program. ablate: {"noedges","nogather","noS","nocc"}."""
    ablate = set(ablate)
    nc = bacc.Bacc("TRN2", target_bir_lowering=False, debug=False,
                   num_devices=NC)
    AF = mybir.ActivationFunctionType
    OP = mybir.AluOpType

    nsc = cfg.nsc
    block, half = cfg.block, cfg.half
    nb = cfg.bins_per_core

    # ---- I/O ----
    x_lo = nc.dram_tensor("x_lo", [half, P], F16, kind="ExternalInput")
    x_hi = nc.dram_tensor("x_hi", [cfg.npad - half, P], F16, kind="ExternalInput")
    iota16_in = nc.dram_tensor("iota16", [P, P], F16, kind="ExternalInput")
    sid_in = nc.dram_tensor("sid", [P, 1], F32, kind="ExternalInput")
    w_in = {}
    for name, shape, dt_ in [
        ("W1", [P, P], F16), ("b1", [P, 1], F32), ("W2", [P, P], F16),
        ("b2", [P, 1], F32), ("fcW1", [P, P // 2], F32), ("fcb1", [P // 2, 1], F32),
        ("gamma", [P // 2, 1], F32), ("beta", [P // 2, 1], F32),
        ("fcW3a", [P // 2 + 1, 1], F32),
    ]:
        w_in[name] = nc.dram_tensor(name, shape, dt_, kind="ExternalInput")
    idx1_in = nc.dram_tensor("idx1", [P, cfg.T1_tot * 8], I16, kind="ExternalInput")
    slot1_in = nc.dram_tensor("slot1", [P, cfg.T1_tot], F32, kind="ExternalInput")
    norm1_in = nc.dram_tensor("norm1", [P, cfg.T1_tot], F32, kind="ExternalInput")
    idx2_in = nc.dram_tensor("idx2", [P, cfg.T2_tot * 8], I16, kind="ExternalInput")
    slot2_in = nc.dram_tensor("slot2", [P, cfg.T2_tot], F32, kind="ExternalInput")
    norm2_in = nc.dram_tensor("norm2", [P, cfg.T2_tot], F32, kind="ExternalInput")
    xblock_in = nc.dram_tensor("xblock", [P, nb, P], F16, kind="ExternalInput")
    snorm_in = nc.dram_tensor("snorm", [P, nb], F32, kind="ExternalInput")
    gid_in = nc.dram_tensor("gid", [P, nb], F32, kind="ExternalInput")
    inv_in = nc.dram_tensor("inv", [P, nb], F32, kind="ExternalInput")
    out_d = nc.dram_tensor("out", [P, 1], F32, kind="ExternalOutput")

    with tile.TileContext(nc) as tc:
        with (
            tc.tile_pool(name="const", bufs=1) as constp,
            tc.tile_pool(name="gbuf", bufs=int(os.environ.get("K_GB", "2")) ) as gbufp,
            tc.tile_pool(name="sp", bufs=int(os.environ.get("K_SP", "6")) ) as sp,
            tc.tile_pool(name="ep", bufs=3) as ep,
            tc.tile_pool(name="psA", bufs=int(os.environ.get("K_PSA", "3")), space="PSUM") as psA,
            tc.tile_pool(name="psH", bufs=2, space="PSUM") as psH,
            tc.tile_pool(name="psT", bufs=2, space="PSUM") as psT,
            tc.tile_pool(name="psPool", bufs=1, space="PSUM") as psPool,
            tc.tile_pool(name="dram", bufs=1, space="DRAM") as dramp,
        ):
            iota16_sb = constp.tile([P, P], F16)
            nc.sync.dma_start(iota16_sb[:], iota16_in[:])
            # layer-1 gather tables first: the first gathers gate the pipeline
            idx1t = constp.tile([P, cfg.T1_tot * 8], I16, name="idx1t")
            nc.sync.dma_start(idx1t[:, 0:64 * 8], idx1_in[:, 0:64 * 8])
            slot1t = constp.tile([P, cfg.T1_tot], F32, name="slot1t")
            nc.sync.dma_start(slot1t[:], slot1_in[:])
            norm1t = constp.tile([P, cfg.T1_tot], F32, name="norm1t")
            nc.sync.dma_start(norm1t[:], norm1_in[:])
            nc.sync.dma_start(idx1t[:, 64 * 8:], idx1_in[:, 64 * 8:])
            ident16 = constp.tile([P, P], F16)
            make_identity(nc, ident16[:])
            ident32 = constp.tile([P, P], F32)
            make_identity(nc, ident32[:])
            iota32_sb = constp.tile([P, P], F32)
            nc.vector.tensor_copy(out=iota32_sb[:], in_=iota16_sb[:])
            sid_sb = constp.tile([P, 1], F32)
            nc.sync.dma_start(sid_sb[:], sid_in[:])
            idx2t = constp.tile([P, cfg.T2_tot * 8], I16, name="idx2t")
            nc.sync.dma_start(idx2t[:], idx2_in[:])
            slot2t = constp.tile([P, cfg.T2_tot], F32, name="slot2t")
            nc.sync.dma_start(slot2t[:], slot2_in[:])
            norm2t = constp.tile([P, cfg.T2_tot], F32, name="norm2t")
            nc.sync.dma_start(norm2t[:], norm2_in[:])
            wsb = {}
            for name, t in w_in.items():
                wsb[name] = constp.tile(list(t.shape), t.dtype, name=f"{name}_sb")
                nc.sync.dma_start(wsb[name][:], t[:])
            xblock_sb = constp.tile([P, nb, P], F16)
            nc.sync.dma_start(xblock_sb[:], xblock_in[:])
            snorm_sb = constp.tile([P, nb], F32)
            nc.sync.dma_start(snorm_sb[:], snorm_in[:])
            gid_sb = constp.tile([P, nb], F32)
            nc.sync.dma_start(gid_sb[:], gid_in[:])
            inv_sb = constp.tile([P, nb], F32)
            nc.sync.dma_start(inv_sb[:], inv_in[:])
            h_n_all = constp.tile([P, nb, P], F16, name="h_n_all")

            for _rep in range(reps):
                # pair-shared h1 table (+1 token row for the barrier)
                h1_pair = dramp.tile([2 * block, P], F16,
                                     addr_space="Shared" if MANUALX else "Local",
                                     name=f"h1_pair{_rep}")
                h1_loc2 = (None if MANUALX else
                           dramp.tile([block, P], F16, name=f"h1_loc2{_rep}"))
                bar_loc = dramp.tile([1, 1], F16, name=f"bar_loc{_rep}")
                bar_out = dramp.tile([2, 1], F16, name=f"bar_out{_rep}")
                partial = [
                    dramp.tile([4, (b - a) * P, 512], F16, name=f"partial{k}_{_rep}")
                    for k, (a, b) in enumerate(cfg.chunks)
                ]
                agg2d = [
                    dramp.tile([(b - a) * P, 512], F16, name=f"agg2_{k}_{_rep}")
                    for k, (a, b) in enumerate(cfg.chunks)
                ]
                pool_loc = dramp.tile([P, P], F32, name=f"pool_loc{_rep}")
                pool_glob = dramp.tile([P, P], F32, addr_space="Shared",
                                       name=f"pool_glob{_rep}")

                pid = nc.sync.partition_id()
                member_off = (pid % 2) * (block * P)

                def cc_issue(*args, **kw):
                    if CCENG == "act":
                        from concourse.bass import BassGpSimd
                        return BassGpSimd.collective_compute(
                            nc.scalar, *args, **kw)
                    return nc.gpsimd.collective_compute(*args, **kw)

                sctr = [0]

                def s_build(slot_ap, norm_ap, name="s_t"):
                    s_t = sp.tile([P, P], F16, name=name)
                    sctr[0] += 1
                    eng = (nc.gpsimd if SFRAC and sctr[0] % SFRAC == 0
                           else nc.vector)
                    eng.tensor_scalar(
                        out=s_t[:], in0=iota16_sb[:], scalar1=slot_ap,
                        scalar2=norm_ap, op0=OP.is_equal, op1=OP.mult)
                    return s_t

                # ================= LAYER 1 =================
                for sc in range(nsc):
                    agg = psA.tile([P, 512], F32, name="agg")
                    first = True
                    for hf in (0, 1):
                        T = int(cfg.T1[sc, hf])
                        off = int(cfg.off1[sc, hf])
                        if "noedges" in ablate:
                            continue
                        g = gbufp.tile([P, T, P], F16, name="g1")
                        if "nogather" not in ablate:
                            for c0 in range(0, T, GCALL):
                                c1 = min(c0 + GCALL, T)
                                nc.gpsimd.dma_gather(
                                    out_ap=g[:, c0:c1, :],
                                    in_ap=(x_lo[:, :] if hf == 0 else x_hi[:, :]),
                                    idxs_ap=idx1t[:, (off + c0) * 8:(off + c1) * 8],
                                    num_idxs=(c1 - c0) * P,
                                    num_idxs_reg=(c1 - c0) * P,
                                    elem_size=P,
                                )
                        else:
                            nc.vector.tensor_copy(out=g[:, 0, :], in_=iota16_sb[:])
                        for t in range(T):
                            sub = int(np.searchsorted(
                                cfg.toff1[sc, hf], t, side="right") - 1)
                            if "noS" not in ablate:
                                s_ap = s_build(slot1t[:, off + t:off + t + 1],
                                               norm1t[:, off + t:off + t + 1])[:]
                            else:
                                s_ap = iota16_sb[:]
                            nc.tensor.matmul(
                                out=agg[:, sub * P:(sub + 1) * P],
                                lhsT=g[:, t if "nogather" not in ablate else 0, :],
                                rhs=s_ap, start=first, stop=False)
                            first = False
                    for sub in range(4):  # self loops: diagonal S, no gather
                        col = sc * 4 + sub
                        sd = s_build(sid_sb[:, 0:1], snorm_sb[:, col:col + 1],
                                     name="sd")
                        nc.tensor.matmul(
                            out=agg[:, sub * P:(sub + 1) * P],
                            lhsT=xblock_sb[:, col, :], rhs=sd[:],
                            start=first, stop=(sub == 3))
                        first = False
                    # epilogue: W1 + relu; node-major h_n; h1_local write
                    aggs = ep.tile([P, 512], F16, name="aggs")
                    nc.scalar.activation(out=aggs[:], in_=agg[:], func=AF.Copy,
                                         bias=0.0, scale=1.0)
                    hps = psH.tile([P, 512], F32, name="hps", tag="h")
                    nc.tensor.matmul(out=hps[:], lhsT=wsb["W1"][:], rhs=aggs[:],
                                     start=True, stop=True)
                    h_t = ep.tile([P, 512], F16, name="h_t")
                    nc.scalar.activation(out=h_t[:], in_=hps[:], func=AF.Relu,
                                         bias=wsb["b1"][:, 0:1], scale=1.0)
                    for sub in range(4):
                        tp = psT.tile([P, P], F16, name="tp", tag="tp")
                        nc.tensor.transpose(
                            tp[:], h_t[:, sub * P:(sub + 1) * P], ident16[:])
                        nc.vector.tensor_copy(out=h_n_all[:, sc * 4 + sub, :], in_=tp[:])
                    if not MANUALX:
                        for (ca, cb) in cfg.agchunks:
                            if sc != cb - 1:
                                continue
                            nc.sync.dma_start(
                                out=h1_loc2[ca * 512:cb * 512, :].rearrange(
                                    "(b p) f -> p b f", p=P),
                                in_=h_n_all[:, ca * 4:cb * 4, :])
                            w = (cb - ca) * 512
                            if "nocc" in ablate:
                                nc.sync.dma_start(
                                    out=h1_pair[ca * 1024:ca * 1024 + w, :],
                                    in_=h1_loc2[ca * 512:cb * 512, :])
                                nc.sync.dma_start(
                                    out=h1_pair[ca * 1024 + w:ca * 1024 + 2 * w, :],
                                    in_=h1_loc2[ca * 512:cb * 512, :])
                            else:
                                cc_issue(
                                    "AllGather", OP.bypass,
                                    replica_groups=[[2 * k, 2 * k + 1]
                                                    for k in range(4)],
                                    ins=[h1_loc2[ca * 512:cb * 512, :]],
                                    outs=[h1_pair[ca * 1024:
                                                  ca * 1024 + 2 * w, :]],
                                )
                # single consolidated write of my whole h1 block into the
                # pair-shared table at my member slot (Shared DRAM requires a
                # single writer instruction)
                if MANUALX:
                    base = h1_pair[0:block, :].rearrange("(b p) f -> p b f", p=P)
                    dstv = AP(base.tensor, base.offset + member_off, base.ap)
                    h1_write = nc.sync.dma_start(out=dstv, in_=h_n_all[:, :, :])
                else:
                    h1_write = None
                # pair barrier: bar_loc rides the same SP DMA ring as the h1
                # write, so its completion implies the write landed; the pair
                # CC syncs both members; the first L2 gather gets an explicit
                # dep on the CC (Pool queue order covers the rest).
                if MANUALX:
                    # data-dep barrier chain: read back a row of my h1 region
                    # (read-after-write on h1_pair), write bar_loc from it,
                    # pair-CC, then write a token into the table's pad row;
                    # every L2 gather reads the whole table incl. that row.
                    rb = ep.tile([1, P], F16, name="rb")
                    rbase = h1_pair[0:1, :]
                    nc.sync.dma_start(
                        out=rb[:, :],
                        in_=AP(rbase.tensor, rbase.offset + member_off,
                               rbase.ap))
                    nc.sync.dma_start(out=bar_loc[:, :], in_=rb[0:1, 0:1])
                    if "nocc" not in ablate:
                        cc_issue(
                            "AllGather", OP.bypass,
                            replica_groups=[[2 * k, 2 * k + 1] for k in range(4)],
                            ins=[bar_loc[:, :]], outs=[bar_out[:, :]],
                        )
                    else:
                        nc.sync.dma_start(out=bar_out[0:1, :], in_=bar_loc[:, :])
                    token16 = ep.tile([1, 1], F16, name="token16")
                    nc.sync.dma_start(out=token16[:], in_=bar_out[0:1, :])
                else:
                    token16 = None

                # ================= LAYER 2 (partials) =================
                pool_ps = psPool.tile([P, P], F32)

                def post_rs(k):
                    ka, kb = cfg.chunks[k]
                    for sc in range(ka, kb):
                        a2 = ep.tile([P, 512], F16, name="a2")
                        nc.sync.dma_start(
                            out=a2[:, :],
                            in_=agg2d[k][(sc - ka) * P:(sc - ka + 1) * P, :])
                        pre = psH.tile([P, 512], F32, name="pre", tag="h")
                        for sub in range(4):
                            col = sc * 4 + sub
                            sd = s_build(sid_sb[:, 0:1], snorm_sb[:, col:col + 1],
                                         name="sd2")
                            nc.tensor.matmul(
                                out=pre[:, sub * P:(sub + 1) * P],
                                lhsT=h_n_all[:, sc * 4 + sub, :], rhs=sd[:],
                                start=(sub == 0), stop=False)
                        nc.tensor.matmul(
                            out=pre[:, 0:512], lhsT=ident16[:],
                            rhs=a2[:, :], start=False, stop=True)
                        fullagg = ep.tile([P, 512], F16, name="fullagg")
                        nc.scalar.activation(out=fullagg[:], in_=pre[:],
                                             func=AF.Copy, bias=0.0, scale=1.0)
                        hps = psH.tile([P, 512], F32, name="hps2", tag="h")
                        nc.tensor.matmul(out=hps[:], lhsT=wsb["W2"][:],
                                         rhs=fullagg[:], start=True, stop=True)
                        h2 = ep.tile([P, 512], F32, name="h2")
                        nc.scalar.activation(out=h2[:], in_=hps[:], func=AF.Relu,
                                             bias=wsb["b2"][:, 0:1], scale=1.0)
                        for sub in range(4):
                            col = sc * 4 + sub
                            tp32 = psT.tile([P, P], F32, name="tp32", tag="tp")
                            nc.tensor.transpose(
                                tp32[:], h2[:, sub * P:(sub + 1) * P], ident32[:])
                            hn32 = sp.tile([P, P], F32, name="hn32")
                            nc.vector.tensor_copy(out=hn32[:], in_=tp32[:])
                            gsel = sp.tile([P, P], F32, name="gsel")
                            nc.vector.tensor_scalar(
                                out=gsel[:], in0=iota32_sb[:],
                                scalar1=gid_sb[:, col:col + 1],
                                scalar2=inv_sb[:, col:col + 1],
                                op0=OP.is_equal, op1=OP.mult,
                            )
                            nc.tensor.matmul(
                                out=pool_ps[:], lhsT=hn32[:], rhs=gsel[:],
                                start=(sc == 0 and sub == 0),
                                stop=(sc == cfg.nsc - 1 and sub == 3))

                for ck, (a, b) in enumerate(cfg.chunks):
                    for sc in range(a, b):
                        T_sc = int(cfg.T2[sc])
                        off = int(cfg.off2[sc])
                        g = None
                        if "noedges" not in ablate:
                            g = gbufp.tile([P, T_sc, P], F16, name="g2")
                            if ck == 0 and sc == a and token16 is not None:
                                # WAW dep: the first gather overwrites this
                                # cell, so it (and, via Pool queue order, all
                                # later gathers) waits for the pair barrier.
                                nc.vector.tensor_copy(out=g[0:1, 0, 0:1],
                                                      in_=token16[:])
                            if "nogather" not in ablate:
                                for c0 in range(0, T_sc, GCALL):
                                    c1 = min(c0 + GCALL, T_sc)
                                    nc.gpsimd.dma_gather(
                                        out_ap=g[:, c0:c1, :],
                                        in_ap=h1_pair[0:2 * block, :],
                                        idxs_ap=idx2t[:, (off + c0) * 8:(off + c1) * 8],
                                        num_idxs=(c1 - c0) * P,
                                        num_idxs_reg=(c1 - c0) * P,
                                        elem_size=P,
                                    )
                            else:
                                nc.vector.tensor_copy(out=g[:, 0, :],
                                                      in_=iota16_sb[:])
                        for j in range(4):
                            agg = psA.tile([P, 512], F32, name="agg")
                            if "noedges" in ablate:
                                nc.tensor.matmul(out=agg[:, 0:512],
                                                 lhsT=ident16[:],
                                                 rhs=xblock_sb[:, 0:4, :].rearrange(
                                                     "p a b -> p (a b)"),
                                                 start=True, stop=True)
                            else:
                                first = True
                                for sub in range(4):
                                    tcnt = int(cfg.t2[sc, j, sub])
                                    t0 = int(cfg.toff2[sc, j, sub])
                                    for t in range(t0, t0 + tcnt):
                                        if "noS" not in ablate:
                                            s_ap = s_build(
                                                slot2t[:, off + t:off + t + 1],
                                                norm2t[:, off + t:off + t + 1])[:]
                                        else:
                                            s_ap = iota16_sb[:]
                                        nc.tensor.matmul(
                                            out=agg[:, sub * P:(sub + 1) * P],
                                            lhsT=g[:, t if "nogather" not in ablate
                                                   else 0, :],
                                            rhs=s_ap, start=first,
                                            stop=(sub == 3 and t == t0 + tcnt - 1))
                                        first = False
                            aggs2 = ep.tile([P, 512], F16, name="aggs2")
                            nc.scalar.activation(out=aggs2[:], in_=agg[:],
                                                 func=AF.Copy, bias=0.0, scale=1.0)
                            nc.sync.dma_start(
                                out=partial[ck][j, (sc - a) * P:(sc - a + 1) * P, :],
                                in_=aggs2[:])
                    # ---- chunk ReduceScatter (delayed by one chunk so the
                    # CC's input wait does not block next-chunk gathers at the
                    # head of the Pool queue) ----
                    def issue_rs(k):
                        if "nocc" in ablate:
                            nc.sync.dma_start(out=agg2d[k][:, :],
                                              in_=partial[k][0, :, :])
                        else:
                            cc_issue(
                                "ReduceScatter", OP.add,
                                replica_groups=[[0, 2, 4, 6], [1, 3, 5, 7]],
                                ins=[partial[k][:, :, :]], outs=[agg2d[k][:, :]],
                            )
                    if RSDELAY:
                        if ck > 0:
                            issue_rs(ck - 1)
                            post_rs(ck - 1)
                        if ck == len(cfg.chunks) - 1:
                            issue_rs(ck)
                            post_rs(ck)
                    else:
                        issue_rs(ck)
                        post_rs(ck)

                # ---- pooled AllReduce + head ----
                pool_sb = ep.tile([P, P], F32, name="pool_sb")
                nc.vector.tensor_copy(out=pool_sb[:], in_=pool_ps[:])
                nc.sync.dma_start(out=pool_loc[:, :], in_=pool_sb[:])
                if "nocc" in ablate:
                    nc.sync.dma_start(out=pool_glob[:, :], in_=pool_loc[:, :])
                else:
                    cc_issue(
                        "AllReduce", OP.add,
                        replica_groups=[list(range(NC))],
                        ins=[pool_loc[:, :]], outs=[pool_glob[:, :]],
                    )
                pooled = ep.tile([P, P], F32, name="pooled")
                nc.sync.dma_start(out=pooled[:], in_=pool_glob[:, :])

                O2 = P // 2
                sm = constp
                zps = psT.tile([O2, P], F32, name="zps", tag="tp")
                nc.tensor.matmul(out=zps[:], lhsT=wsb["fcW1"][:], rhs=pooled[:],
                                 start=True, stop=True)
                z = ep.tile([O2, P], F32, name="z")
                nc.scalar.activation(out=z[:], in_=zps[:], func=AF.Relu,
                                     bias=wsb["fcb1"][:, 0:1], scale=1.0)
                mu = sm.tile([O2, 1], F32, name="mu")
                nc.vector.tensor_reduce(out=mu[:], in_=z[:],
                                        axis=mybir.AxisListType.X, op=OP.add)
                sq = sm.tile([O2, P], F32, name="sq")
                nc.vector.tensor_tensor(out=sq[:], in0=z[:], in1=z[:], op=OP.mult)
                s2m = sm.tile([O2, 1], F32, name="s2m")
                nc.vector.tensor_reduce(out=s2m[:], in_=sq[:],
                                        axis=mybir.AxisListType.X, op=OP.add)
                mu_m = sm.tile([O2, 1], F32, name="mu_m")
                nc.vector.tensor_scalar_mul(mu_m[:], mu[:], 1.0 / NUM_GRAPHS)
                ex2 = sm.tile([O2, 1], F32, name="ex2")
                nc.vector.tensor_scalar_mul(ex2[:], s2m[:], 1.0 / NUM_GRAPHS)
                musq = sm.tile([O2, 1], F32, name="musq")
                nc.vector.tensor_tensor(out=musq[:], in0=mu_m[:], in1=mu_m[:],
                                        op=OP.mult)
                var = sm.tile([O2, 1], F32, name="var")
                nc.vector.tensor_tensor(out=var[:], in0=ex2[:], in1=musq[:],
                                        op=OP.subtract)
                varep = sm.tile([O2, 1], F32, name="varep")
                nc.vector.tensor_scalar_add(varep[:], var[:], BN_EPS)
                sd_ = sm.tile([O2, 1], F32, name="sd_")
                nc.scalar.activation(out=sd_[:], in_=varep[:], func=AF.Sqrt,
                                     bias=0.0, scale=1.0)
                rstd = sm.tile([O2, 1], F32, name="rstd")
                nc.vector.reciprocal(out=rstd[:], in_=sd_[:])
                seff = sm.tile([O2, 1], F32, name="seff")
                nc.vector.tensor_tensor(out=seff[:], in0=rstd[:],
                                        in1=wsb["gamma"][:], op=OP.mult)
                tmp = sm.tile([O2, 1], F32, name="tmp")
                nc.vector.tensor_tensor(out=tmp[:], in0=mu_m[:], in1=seff[:],
                                        op=OP.mult)
                beff = sm.tile([O2, 1], F32, name="beff")
                nc.vector.tensor_tensor(out=beff[:], in0=wsb["beta"][:],
                                        in1=tmp[:], op=OP.subtract)
                zaug = sm.tile([O2 + 1, P], F32, name="zaug")
                nc.vector.tensor_scalar(out=zaug[0:O2, :], in0=z[:],
                                        scalar1=seff[:, 0:1], scalar2=beff[:, 0:1],
                                        op0=OP.mult, op1=OP.add)
                nc.gpsimd.memset(zaug[O2:O2 + 1, :], 1.0)
                fin_ps = psT.tile([P, 1], F32, name="fin_ps", tag="tp")
                nc.tensor.matmul(out=fin_ps[:], lhsT=zaug[:, :],
                                 rhs=wsb["fcW3a"][:, :], start=True, stop=True)
                fin_sb = sm.tile([P, 1], F32, name="fin_sb")
                nc.vector.tensor_copy(out=fin_sb[:], in_=fin_ps[:])
                nc.sync.dma_start(out=out_d[:, :], in_=fin_sb[:])

    nc.compile()
    return nc


def _make_in_maps(cfg, shared, percore, weights):
    in_maps = []
    for c in range(NC):
        m = dict(shared)
        for k in ("idx1", "slot1", "norm1", "idx2", "slot2", "norm2",
                  "xblock", "snorm", "gid", "inv"):
            m[k] = percore[k][c]
        m.update(weights)
        in_maps.append(m)
    return in_maps


def _weights_arrays(W1, b1, W2, b2, fcW1, fcb1, gamma, beta, fcW3, fcb3):
    f = np.float32
    return {
        "W1": np.ascontiguousarray(np.asarray(W1, f).astype(np.float16)),
        "b1": np.ascontiguousarray(np.asarray(b1, f).reshape(-1, 1)),
        "W2": np.ascontiguousarray(np.asarray(W2, f).astype(np.float16)),
        "b2": np.ascontiguousarray(np.asarray(b2, f).reshape(-1, 1)),
        "fcW1": np.ascontiguousarray(fcW1, f),
        "fcb1": np.ascontiguousarray(np.asarray(fcb1, f).reshape(-1, 1)),
        "gamma": np.ascontiguousarray(np.asarray(gamma, f).reshape(-1, 1)),
        "beta": np.ascontiguousarray(np.asarray(beta, f).reshape(-1, 1)),
        "fcW3a": np.ascontiguousarray(
            np.concatenate([np.asarray(fcW3, f).reshape(-1, 1),
                            np.asarray(fcb3, f).reshape(1, 1)], axis=0)),
    }


def _pjrt_bench(nc, in_maps, n_cores, iters=20):
    """Keeps inputs device-resident; times steady-state executions."""
    import time

    import jax
    from jax.experimental.shard_map import shard_map
    from jax.sharding import Mesh, NamedSharding, PartitionSpec

    from concourse import bass2jax

    bass2jax.install_neuronx_cc_hook()
    partition_name = nc.partition_id_tensor.name if nc.partition_id_tensor else None
    in_names, out_names, out_avals, zero_outs = [], [], [], []
    for alloc in nc.m.functions[0].allocations:
        if not isinstance(alloc, mybir.MemoryLocationSet):
            continue
        name = alloc.memorylocations[0].name
        if alloc.kind == "ExternalInput":
            if name != partition_name:
                in_names.append(name)
        elif alloc.kind == "ExternalOutput":
            out_names.append(name)
            shape = tuple(alloc.tensor_shape)
            dtype = mybir.dt.np(alloc.dtype)
            out_avals.append(jax.core.ShapedArray(shape, dtype))
            zero_outs.append(np.zeros(shape, dtype))
    n_params = len(in_names)
    n_outs = len(out_avals)
    in_names_all = list(in_names) + out_names
    if partition_name is not None:
        in_names_all.append(partition_name)

    def _body(*args):
        operands = list(args)
        if partition_name is not None:
            operands.append(bass2jax.partition_id_tensor())
        outs = bass2jax._bass_exec_p.bind(
            *operands,
            out_avals=tuple(out_avals),
            in_names=tuple(in_names_all),
            out_names=tuple(out_names),
            lowering_input_output_aliases=(),
            sim_require_finite=True,
            sim_require_nnan=True,
            nc=nc,
        )
        return tuple(outs)

    devices = jax.devices()[:n_cores]
    mesh = Mesh(np.asarray(devices), ("core",))
    donate = tuple(range(n_params, n_params + n_outs))
    sharded = jax.jit(
        shard_map(_body, mesh=mesh,
                  in_specs=(PartitionSpec("core"),) * (n_params + n_outs),
                  out_specs=(PartitionSpec("core"),) * n_outs, check_rep=False),
        donate_argnums=donate, keep_unused=True,
    )
    spec = NamedSharding(mesh, PartitionSpec("core"))
    concat_in = [
        jax.device_put(
            np.concatenate([np.asarray(in_maps[c][nm]) for c in range(n_cores)],
                           axis=0), spec)
        for nm in in_names
    ]
    for a in concat_in:
        a.block_until_ready()

    def zeros():
        return [np.zeros((n_cores * z.shape[0], *z.shape[1:]), z.dtype)
                for z in zero_outs]

    out_arrs = sharded(*concat_in, *zeros())
    jax.block_until_ready(out_arrs)
    results = [
        {nm: np.asarray(out_arrs[i]).reshape(n_cores, *out_avals[i].shape)[c]
         for i, nm in enumerate(out_names)}
        for c in range(n_cores)
    ]
    import time as _t
    t0 = _t.perf_counter()
    last = None
    for _ in range(iters):
        last = sharded(*concat_in, *zeros())
    jax.block_until_ready(last)
    per_iter_ns = (_t.perf_counter() - t0) / iters * 1e9
    return results, per_iter_ns


def run(inputs, trace=False, nc_cores=8):
    cfg, shared, percore = _prep(inputs["x"], inputs["edge_index"],
                                 inputs["batch"], nc_cores=nc_cores)
    weights = _weights_arrays(
        inputs["W1"], inputs["b1"], inputs["W2"], inputs["b2"],
        inputs["fcW1"], inputs["fcb1"], inputs["gamma"], inputs["beta"],
        inputs["fcW3"], inputs["fcb3"])
    nc = _build(cfg)
    in_maps = _make_in_maps(cfg, shared, percore, weights)
    if trace:
        results, per_iter_ns = _pjrt_bench(nc, in_maps, cfg.nc)
        out = np.asarray(results[0]["out"], np.float32).reshape(NUM_GRAPHS, 1)
        return out, per_iter_ns
    res = run_bass_kernel_spmd(nc, in_maps, list(range(cfg.nc)), trace=False)
    out = np.asarray(res.results[0]["out"], np.float32).reshape(NUM_GRAPHS, 1)
    return out, res.exec_time_ns


def kernel(**inputs) -> np.ndarray:
    out, _ = run(inputs, trace=False)
    return out


# revision 50
# speedup vs baseline: 1.3468x; 1.3468x over previous
"""Trainium2 Bass kernel for AffinityNet (2-layer GCN + mean-pool + MLP head).

v2 strategy (8 NeuronCores, SPMD):
  - Node slots: 8 cores x 13 superchunks x 512 slots (npad=53248), balanced
    in-degree bin packing.
  - Layer 1: dst-partitioned. Per (sc, half): dma_gather x rows (fp16, table
    split in 2 halves for int16 idx), one-hot S on DVE (fp16 iota), PE
    accumulates agg^T[f, slot] in PSUM. Self-loops via diagonal-S matmuls on
    the SBUF-resident x block (no gather). Then W1 matmul + ReLU; node-major
    h_n kept in SBUF; h1 block written to local DRAM.
  - h1 exchange: ONE pair AllGather ([[0,1],[2,3],..]) -> h1_pair [2*block]
    (pair cores share HBM, so this is cheap on HW).
  - Layer 2: src-PAIR-partitioned, target-parity split: core c handles edges
    with src in its pair and dst core ≡ c (mod 2). Gathers read h1_pair (one
    table, no halves). Partial aggregations agg^T in fp16 -> partial[tc_slot,
    sc] DRAM, 3 sc-chunks; dual 4-core ReduceScatter ([[0,2,4,6],[1,3,5,7]])
    delivers summed agg2 for own slots. Self-loop + bias + W2 + ReLU + pool
    matmul epilogue per sc; pool AllReduce (64KB) + replicated MLP head.
"""

import sys

sys.path.insert(0, "/opt/trn_rl_repo")

import math

import numpy as np

from concourse import bacc, mybir, tile
from concourse.bass_types import AP
from concourse.bass_utils import run_bass_kernel_spmd
from concourse.masks import make_identity

F32 = mybir.dt.float32
F16 = mybir.dt.float16
I16 = mybir.dt.int16
P = 128
NC = 8
NUM_GRAPHS = 128
BN_EPS = 1e-5
import os

GCALL = int(os.environ.get("K_GCALL", "8"))  # tiles per dma_gather call (1024 idx HW cap)
AGCH = 13 if int(os.environ.get("K_MANUALX", "0")) else int(os.environ.get("K_AGCH", "13"))  # AG chunk size (sc)
SFRAC = int(os.environ.get("K_SFRAC", "0"))   # 1/SFRAC S-builds on gpsimd
RSDELAY = int(os.environ.get("K_RSDELAY", "1"))  # delay RS issue by one chunk
MANUALX = int(os.environ.get("K_MANUALX", "0"))  # manual pair h1 exchange
CCENG = os.environ.get("K_CCENG", "pool")  # engine queue for collectives


class Cfg:
    def __init__(self, n, nsc, t1, t2):
        self.n = n
        self.nc = NC
        self.nsc = nsc
        self.t1 = t1  # [nsc, 2, 4] tiles per (sc, half, sub), shared
        self.t2 = t2  # [nsc, 4, 4] tiles per (sc, j, sub), shared
        self.bins_per_core = nsc * 4
        self.block = self.bins_per_core * P
        self.npad = NC * self.block
        self.half = self.npad // 2
        # L1 layout: segments (sc, half); within: sub-major tiles
        self.T1 = t1.sum(axis=2)          # [nsc, 2]
        self.off1 = np.concatenate([[0], np.cumsum(self.T1.reshape(-1))])[
            :-1].reshape(self.T1.shape)   # [nsc, 2] global tile offset
        self.T1_tot = int(self.T1.sum())
        self.toff1 = np.cumsum(t1, axis=2) - t1  # [nsc,2,4] within-seg offset
        # L2 layout: segments (sc); within: (j, sub)-major tiles
        self.T2 = t2.sum(axis=(1, 2))     # [nsc]
        self.off2 = np.concatenate([[0], np.cumsum(self.T2)])[:-1]
        self.T2_tot = int(self.T2.sum())
        t2f = t2.reshape(self.nsc, 16)
        self.toff2 = (np.cumsum(t2f, axis=1) - t2f).reshape(t2.shape)
        # RS sc-chunks
        import os as _os
        _l = _os.environ.get("K_RSCHL", "6,5,2" if nsc == 13 else "")
        if _l:
            ch = [int(x) for x in _l.replace(";", ",").split(",")]
            assert sum(ch) == nsc
        else:
            nch = int(_os.environ.get("K_RSCH", "3"))
            ch = [nsc // nch + (1 if i < nsc % nch else 0) for i in range(nch)]
        self.chunks = []
        s = 0
        for c in ch:
            self.chunks.append((s, s + c))
            s += c
        # pair-AllGather sc-chunks
        self.agchunks = [(a, min(a + AGCH, nsc)) for a in range(0, nsc, AGCH)]

    def __repr__(self):
        return (f"Cfg(n={self.n}, nsc={self.nsc}, T1_tot={self.T1_tot}, "
                f"T2_tot={self.T2_tot}, chunks={self.chunks})")


def _pack_bins(deg, nbins, cap):
    import heapq

    n = len(deg)
    order = np.argsort(-deg, kind="stable")
    heap = [(0.0, b) for b in range(nbins)]
    heapq.heapify(heap)
    fill = np.zeros(nbins, np.int64)
    rowof = np.empty(n, np.int64)
    for node in order:
        while True:
            load, b = heapq.heappop(heap)
            if fill[b] < cap:
                break
        rowof[node] = b * cap + fill[b]
        fill[b] += 1
        if fill[b] < cap:
            heapq.heappush(heap, (load + float(deg[node]), b))
    return rowof


def _wrap_seg(flat):
    """[nc, S] int16 (S % 128 == 0) -> [nc, 128, S//16] gather idx layout."""
    nc_, s = flat.shape
    a = flat.reshape(nc_, s // 16, 16)
    a = np.swapaxes(a, -1, -2)  # [nc, 16, S//16]
    return np.tile(a, (1, 8, 1)).astype(np.int16)


def _pos_in_group(key, ngroups):
    cnts = np.bincount(key, minlength=ngroups)
    order = np.argsort(key, kind="stable")
    starts = np.zeros(ngroups, np.int64)
    starts[1:] = np.cumsum(cnts)[:-1]
    pos = np.empty(len(key), np.int64)
    pos[order] = np.arange(len(key)) - starts[key[order]]
    return pos, cnts


def _prep(x, edge_index, batch, nc_cores=8):
    assert nc_cores == NC
    x = np.ascontiguousarray(np.asarray(x, np.float32))
    edge_index = np.asarray(edge_index)
    batch = np.asarray(batch).astype(np.int64)
    n, f = x.shape
    assert f == P

    src = edge_index[0].astype(np.int64)
    dst = edge_index[1].astype(np.int64)

    deg = np.bincount(dst, minlength=n).astype(np.float64) + 1.0
    dinv = (1.0 / np.sqrt(deg)).astype(np.float32)
    snorm_node = (dinv * dinv).astype(np.float32)

    nbins_needed = math.ceil(n / P)
    bins_per_core = math.ceil(nbins_needed / (NC * 4)) * 4
    nsc = bins_per_core // 4
    nbins = NC * bins_per_core
    rowof = _pack_bins(deg, nbins, P)
    block = bins_per_core * P
    npad = nbins * P
    half = npad // 2
    assert half <= 32767 and 2 * block <= 32768

    drow = rowof[dst]
    srow = rowof[src]
    norm_e = (dinv[src] * dinv[dst]).astype(np.float32)

    core1 = drow // block
    sc_d = (drow % block) // 512
    sub_d = (drow % 512) // P
    slot_d = (drow % P).astype(np.float32)

    # ---- L1: dst-partitioned, src idx into x table halves ----
    halfe = (srow >= half).astype(np.int64)
    idx1v = (srow - halfe * half).astype(np.int64)
    key1 = ((core1 * nsc + sc_d) * 2 + halfe) * 4 + sub_d
    pos1, cnts1 = _pos_in_group(key1, NC * nsc * 2 * 4)
    c1 = cnts1.reshape(NC, nsc, 2, 4)
    t1 = np.ceil(c1.max(axis=0) / P).astype(np.int64)  # [nsc,2,4]
    t1 = np.maximum(t1, 1)

    # ---- L2: src-pair partitioned, dst parity split ----
    pair_s = srow // (2 * block)
    # h1_pair interleaved layout: per AG chunk of AGCH superchunks,
    # [member, sc-in-chunk, 512] blocks (member-major within the chunk)
    scs_s = (srow % block) // 512
    member_s = (srow // block) % 2
    scp_s = scs_s // AGCH
    chunk_nsc = np.minimum(AGCH, nsc - scp_s * AGCH)
    lrow2 = (scp_s * AGCH * 1024 + member_s * chunk_nsc * 512
             + (scs_s % AGCH) * 512 + (srow % 512))
    parity = core1 % 2
    j2 = core1 // 2
    pcore = pair_s * 2 + parity
    key2 = ((pcore * nsc + sc_d) * 4 + j2) * 4 + sub_d
    pos2, cnts2 = _pos_in_group(key2, NC * nsc * 4 * 4)
    c2 = cnts2.reshape(NC, nsc, 4, 4)
    t2 = np.ceil(c2.max(axis=0) / P).astype(np.int64)  # [nsc, 4(j), 4(sub)]
    t2 = np.maximum(t2, 1)

    cfg = Cfg(n=n, nsc=nsc, t1=t1, t2=t2)

    # ---- L1 arrays ----
    s1 = (cfg.off1[sc_d, halfe] + cfg.toff1[sc_d, halfe, sub_d]) * P + pos1
    idx1_f = np.zeros((NC, cfg.T1_tot * P), np.int64)
    slot1_f = np.zeros((NC, cfg.T1_tot * P), np.float32)
    norm1_f = np.zeros((NC, cfg.T1_tot * P), np.float32)
    idx1_f[core1, s1] = idx1v
    slot1_f[core1, s1] = slot_d
    norm1_f[core1, s1] = norm_e
    idx1_w = _wrap_seg(idx1_f.astype(np.int16))
    slot1_w = np.swapaxes(slot1_f.reshape(NC, cfg.T1_tot, P), -1, -2).copy()
    norm1_w = np.swapaxes(norm1_f.reshape(NC, cfg.T1_tot, P), -1, -2).copy()

    # ---- L2 arrays ----
    s2 = (cfg.off2[sc_d] + cfg.toff2[sc_d, j2, sub_d]) * P + pos2
    idx2_f = np.zeros((NC, cfg.T2_tot * P), np.int64)
    slot2_f = np.zeros((NC, cfg.T2_tot * P), np.float32)
    norm2_f = np.zeros((NC, cfg.T2_tot * P), np.float32)
    idx2_f[pcore, s2] = lrow2
    slot2_f[pcore, s2] = slot_d
    norm2_f[pcore, s2] = norm_e
    idx2_w = _wrap_seg(idx2_f.astype(np.int16))
    slot2_w = np.swapaxes(slot2_f.reshape(NC, cfg.T2_tot, P), -1, -2).copy()
    norm2_w = np.swapaxes(norm2_f.reshape(NC, cfg.T2_tot, P), -1, -2).copy()

    # ---- node-slot statics ----
    x_perm = np.zeros((npad, P), np.float16)
    x_perm[rowof] = x.astype(np.float16)
    xblock_w = x_perm.reshape(NC, bins_per_core, P, P).transpose(0, 2, 1, 3).copy()
    snorm_full = np.zeros(npad, np.float32)
    snorm_full[rowof] = snorm_node
    snorm_w = snorm_full.reshape(NC, bins_per_core, P).transpose(0, 2, 1).copy()

    cnt_g = np.bincount(batch, minlength=NUM_GRAPHS).astype(np.float32)
    invc = 1.0 / np.maximum(cnt_g, 1.0)
    gid_full = -np.ones(npad, np.float32)
    inv_full = np.zeros(npad, np.float32)
    gid_full[rowof] = batch.astype(np.float32)
    inv_full[rowof] = invc[batch]
    gid_w = gid_full.reshape(NC, bins_per_core, P).transpose(0, 2, 1).copy()
    inv_w = inv_full.reshape(NC, bins_per_core, P).transpose(0, 2, 1).copy()

    shared = {
        "x_lo": np.ascontiguousarray(x_perm[:half]),
        "x_hi": np.ascontiguousarray(x_perm[half:]),
        "iota16": np.tile(np.arange(P, dtype=np.float16), (P, 1)),
        "sid": np.arange(P, dtype=np.float32).reshape(P, 1),
    }
    percore = {
        "idx1": idx1_w, "slot1": slot1_w, "norm1": norm1_w,
        "idx2": idx2_w, "slot2": slot2_w, "norm2": norm2_w,
        "xblock": xblock_w, "snorm": snorm_w, "gid": gid_w, "inv": inv_w,
    }
    return cfg, shared, percore


def _build(cfg, ablate=(), reps=1):
    """Build the SPMD Bass program. ablate: {"noedges","nogather","noS","nocc"}."""
    ablate = set(ablate)
    nc = bacc.Bacc("TRN2", target_bir_lowering=False, debug=False,
                   num_devices=NC)
    AF = mybir.ActivationFunctionType
    OP = mybir.AluOpType

    nsc = cfg.nsc
    block, half = cfg.block, cfg.half
    nb = cfg.bins_per_core

    # ---- I/O ----
    x_lo = nc.dram_tensor("x_lo", [half, P], F16, kind="ExternalInput")
    x_hi = nc.dram_tensor("x_hi", [cfg.npad - half, P], F16, kind="ExternalInput")
    iota16_in = nc.dram_tensor("iota16", [P, P], F16, kind="ExternalInput")
    sid_in = nc.dram_tensor("sid", [P, 1], F32, kind="ExternalInput")
    w_in = {}
    for name, shape, dt_ in [
        ("W1", [P, P], F16), ("b1", [P, 1], F32), ("W2", [P, P], F16),
        ("b2", [P, 1], F32), ("fcW1", [P, P // 2], F32), ("fcb1", [P // 2, 1], F32),
        ("gamma", [P // 2, 1], F32), ("beta", [P // 2, 1], F32),
        ("fcW3a", [P // 2 + 1, 1], F32),
    ]:
        w_in[name] = nc.dram_tensor(name, shape, dt_, kind="ExternalInput")
    idx1_in = nc.dram_tensor("idx1", [P, cfg.T1_tot * 8], I16, kind="ExternalInput")
    slot1_in = nc.dram_tensor("slot1", [P, cfg.T1_tot], F32, kind="ExternalInput")
    norm1_in = nc.dram_tensor("norm1", [P, cfg.T1_tot], F32, kind="ExternalInput")
    idx2_in = nc.dram_tensor("idx2", [P, cfg.T2_tot * 8], I16, kind="ExternalInput")
    slot2_in = nc.dram_tensor("slot2", [P, cfg.T2_tot], F32, kind="ExternalInput")
    norm2_in = nc.dram_tensor("norm2", [P, cfg.T2_tot], F32, kind="ExternalInput")
    xblock_in = nc.dram_tensor("xblock", [P, nb, P], F16, kind="ExternalInput")
    snorm_in = nc.dram_tensor("snorm", [P, nb], F32, kind="ExternalInput")
    gid_in = nc.dram_tensor("gid", [P, nb], F32, kind="ExternalInput")
    inv_in = nc.dram_tensor("inv", [P, nb], F32, kind="ExternalInput")
    out_d = nc.dram_tensor("out", [P, 1], F32, kind="ExternalOutput")

    with tile.TileContext(nc) as tc:
        with (
            tc.tile_pool(name="const", bufs=1) as constp,
            tc.tile_pool(name="gbuf", bufs=int(os.environ.get("K_GB", "2")) ) as gbufp,
            tc.tile_pool(name="sp", bufs=int(os.environ.get("K_SP", "6")) ) as sp,
            tc.tile_pool(name="ep", bufs=3) as ep,
            tc.tile_pool(name="psA", bufs=int(os.environ.get("K_PSA", "3")), space="PSUM") as psA,
            tc.tile_pool(name="psH", bufs=2, space="PSUM") as psH,
            tc.tile_pool(name="psT", bufs=2, space="PSUM") as psT,
            tc.tile_pool(name="psPool", bufs=1, space="PSUM") as psPool,
            tc.tile_pool(name="dram", bufs=1, space="DRAM") as dramp,
        ):
            iota16_sb = constp.tile([P, P], F16)
            nc.sync.dma_start(iota16_sb[:], iota16_in[:])
            # layer-1 gather tables first: the first gathers gate the pipeline
            idx1t = constp.tile([P, cfg.T1_tot * 8], I16, name="idx1t")
            nc.sync.dma_start(idx1t[:, 0:64 * 8], idx1_in[:, 0:64 * 8])
            slot1t = constp.tile([P, cfg.T1_tot], F32, name="slot1t")
            nc.sync.dma_start(slot1t[:], slot1_in[:])
            norm1t = constp.tile([P, cfg.T1_tot], F32, name="norm1t")
            nc.sync.dma_start(norm1t[:], norm1_in[:])
            nc.sync.dma_start(idx1t[:, 64 * 8:], idx1_in[:, 64 * 8:])
            ident16 = constp.tile([P, P], F16)
            make_identity(nc, ident16[:])
            ident32 = constp.tile([P, P], F32)
            make_identity(nc, ident32[:])
            iota32_sb = constp.tile([P, P], F32)
            nc.vector.tensor_copy(out=iota32_sb[:], in_=iota16_sb[:])
            sid_sb = constp.tile([P, 1], F32)
            nc.sync.dma_start(sid_sb[:], sid_in[:])
            idx2t = constp.tile([P, cfg.T2_tot * 8], I16, name="idx2t")
            nc.sync.dma_start(idx2t[:], idx2_in[:])
            slot2t = constp.tile([P, cfg.T2_tot], F32, name="slot2t")
            nc.sync.dma_start(slot2t[:], slot2_in[:])
            norm2t = constp.tile([P, cfg.T2_tot], F32, name="norm2t")
            nc.sync.dma_start(norm2t[:], norm2_in[:])
            wsb = {}
            for name, t in w_in.items():
                wsb[name] = constp.tile(list(t.shape), t.dtype, name=f"{name}_sb")
                nc.sync.dma_start(wsb[name][:], t[:])
            xblock_sb = constp.tile([P, nb, P], F16)
            nc.sync.dma_start(xblock_sb[:], xblock_in[:])
            snorm_sb = constp.tile([P, nb], F32)
            nc.sync.dma_start(snorm_sb[:], snorm_in[:])
            gid_sb = constp.tile([P, nb], F32)
            nc.sync.dma_start(gid_sb[:], gid_in[:])
            inv_sb = constp.tile([P, nb], F32)
            nc.sync.dma_start(inv_sb[:], inv_in[:])
            h_n_all = constp.tile([P, nb, P], F16, name="h_n_all")

            for _rep in range(reps):
                # pair-shared h1 table (+1 token row for the barrier)
                h1_pair = dramp.tile([2 * block, P], F16,
                                     addr_space="Shared" if MANUALX else "Local",
                                     name=f"h1_pair{_rep}")
                h1_loc2 = (None if MANUALX else
                           dramp.tile([block, P], F16, name=f"h1_loc2{_rep}"))
                bar_loc = dramp.tile([1, 1], F16, name=f"bar_loc{_rep}")
                bar_out = dramp.tile([2, 1], F16, name=f"bar_out{_rep}")
                partial = [
                    dramp.tile([4, (b - a) * P, 512], F16, name=f"partial{k}_{_rep}")
                    for k, (a, b) in enumerate(cfg.chunks)
                ]
                agg2d = [
                    dramp.tile([(b - a) * P, 512], F16, name=f"agg2_{k}_{_rep}")
                    for k, (a, b) in enumerate(cfg.chunks)
                ]
                pool_loc = dramp.tile([P, P], F32, name=f"pool_loc{_rep}")
                pool_glob = dramp.tile([P, P], F32, addr_space="Shared",
                                       name=f"pool_glob{_rep}")

                pid = nc.sync.partition_id()
                member_off = (pid % 2) * (block * P)

                def cc_issue(*args, **kw):
                    if CCENG == "act":
                        from concourse.bass import BassGpSimd
                        return BassGpSimd.collective_compute(
                            nc.scalar, *args, **kw)
                    return nc.gpsimd.collective_compute(*args, **kw)

                sctr = [0]

                def s_build(slot_ap, norm_ap, name="s_t"):
                    s_t = sp.tile([P, P], F16, name=name)
                    sctr[0] += 1
                    eng = (nc.gpsimd if SFRAC and sctr[0] % SFRAC == 0
                           else nc.vector)
                    eng.tensor_scalar(
                        out=s_t[:], in0=iota16_sb[:], scalar1=slot_ap,
                        scalar2=norm_ap, op0=OP.is_equal, op1=OP.mult)
                    return s_t

                # ================= LAYER 1 =================
                for sc in range(nsc):
                    agg = psA.tile([P, 512], F32, name="agg")
                    first = True
                    for hf in (0, 1):
                        T = int(cfg.T1[sc, hf])
                        off = int(cfg.off1[sc, hf])
                        if "noedges" in ablate:
                            continue
                        g = gbufp.tile([P, T, P], F16, name="g1")
                        if "nogather" not in ablate:
                            for c0 in range(0, T, GCALL):
                                c1 = min(c0 + GCALL, T)
                                nc.gpsimd.dma_gather(
                                    out_ap=g[:, c0:c1, :],
                                    in_ap=(x_lo[:, :] if hf == 0 else x_hi[:, :]),
                                    idxs_ap=idx1t[:, (off + c0) * 8:(off + c1) * 8],
                                    num_idxs=(c1 - c0) * P,
                                    num_idxs_reg=(c1 - c0) * P,
                                    elem_size=P,
                                )
                        else:
                            nc.vector.tensor_copy(out=g[:, 0, :], in_=iota16_sb[:])
                        for t in range(T):
                            sub = int(np.searchsorted(
                                cfg.toff1[sc, hf], t, side="right") - 1)
                            if "noS" not in ablate:
                                s_ap = s_build(slot1t[:, off + t:off + t + 1],
                                               norm1t[:, off + t:off + t + 1])[:]
                            else:
                                s_ap = iota16_sb[:]
                            nc.tensor.matmul(
                                out=agg[:, sub * P:(sub + 1) * P],
                                lhsT=g[:, t if "nogather" not in ablate else 0, :],
                                rhs=s_ap, start=first, stop=False)
                            first = False
                    for sub in range(4):  # self loops: diagonal S, no gather
                        col = sc * 4 + sub
                        sd = s_build(sid_sb[:, 0:1], snorm_sb[:, col:col + 1],
                                     name="sd")
                        nc.tensor.matmul(
                            out=agg[:, sub * P:(sub + 1) * P],
                            lhsT=xblock_sb[:, col, :], rhs=sd[:],
                            start=first, stop=(sub == 3))
                        first = False
                    # epilogue: W1 + relu; node-major h_n; h1_local write
                    aggs = ep.tile([P, 512], F16, name="aggs")
                    nc.scalar.activation(out=aggs[:], in_=agg[:], func=AF.Copy,
                                         bias=0.0, scale=1.0)
                    hps = psH.tile([P, 512], F32, name="hps", tag="h")
                    nc.tensor.matmul(out=hps[:], lhsT=wsb["W1"][:], rhs=aggs[:],
                                     start=True, stop=True)
                    h_t = ep.tile([P, 512], F16, name="h_t")
                    nc.scalar.activation(out=h_t[:], in_=hps[:], func=AF.Relu,
                                         bias=wsb["b1"][:, 0:1], scale=1.0)
                    for sub in range(4):
                        tp = psT.tile([P, P], F16, name="tp", tag="tp")
                        nc.tensor.transpose(
                            tp[:], h_t[:, sub * P:(sub + 1) * P], ident16[:])
                        nc.vector.tensor_copy(out=h_n_all[:, sc * 4 + sub, :], in_=tp[:])
                    if not MANUALX:
                        # stage this superchunk's h1 rows immediately so the
                        # AllGather's input is ready right after the last one
                        nc.sync.dma_start(
                            out=h1_loc2[sc * 512:(sc + 1) * 512, :].rearrange(
                                "(b p) f -> p b f", p=P),
                            in_=h_n_all[:, sc * 4:(sc + 1) * 4, :])
                        for (ca, cb) in cfg.agchunks:
                            if sc != cb - 1:
                                continue
                            w = (cb - ca) * 512
                            if "nocc" in ablate:
                                nc.sync.dma_start(
                                    out=h1_pair[ca * 1024:ca * 1024 + w, :],
                                    in_=h1_loc2[ca * 512:cb * 512, :])
                                nc.sync.dma_start(
                                    out=h1_pair[ca * 1024 + w:ca * 1024 + 2 * w, :],
                                    in_=h1_loc2[ca * 512:cb * 512, :])
                            else:
                                cc_issue(
                                    "AllGather", OP.bypass,
                                    replica_groups=[[2 * k, 2 * k + 1]
                                                    for k in range(4)],
                                    ins=[h1_loc2[ca * 512:cb * 512, :]],
                                    outs=[h1_pair[ca * 1024:
                                                  ca * 1024 + 2 * w, :]],
                                )
                # single consolidated write of my whole h1 block into the
                # pair-shared table at my member slot (Shared DRAM requires a
                # single writer instruction)
                if MANUALX:
                    base = h1_pair[0:block, :].rearrange("(b p) f -> p b f", p=P)
                    dstv = AP(base.tensor, base.offset + member_off, base.ap)
                    h1_write = nc.sync.dma_start(out=dstv, in_=h_n_all[:, :, :])
                else:
                    h1_write = None
                # pair barrier: bar_loc rides the same SP DMA ring as the h1
                # write, so its completion implies the write landed; the pair
                # CC syncs both members; the first L2 gather gets an explicit
                # dep on the CC (Pool queue order covers the rest).
                if MANUALX:
                    # data-dep barrier chain: read back a row of my h1 region
                    # (read-after-write on h1_pair), write bar_loc from it,
                    # pair-CC, then write a token into the table's pad row;
                    # every L2 gather reads the whole table incl. that row.
                    rb = ep.tile([1, P], F16, name="rb")
                    rbase = h1_pair[0:1, :]
                    nc.sync.dma_start(
                        out=rb[:, :],
                        in_=AP(rbase.tensor, rbase.offset + member_off,
                               rbase.ap))
                    nc.sync.dma_start(out=bar_loc[:, :], in_=rb[0:1, 0:1])
                    if "nocc" not in ablate:
                        cc_issue(
                            "AllGather", OP.bypass,
                            replica_groups=[[2 * k, 2 * k + 1] for k in range(4)],
                            ins=[bar_loc[:, :]], outs=[bar_out[:, :]],
                        )
                    else:
                        nc.sync.dma_start(out=bar_out[0:1, :], in_=bar_loc[:, :])
                    token16 = ep.tile([1, 1], F16, name="token16")
                    nc.sync.dma_start(out=token16[:], in_=bar_out[0:1, :])
                else:
                    token16 = None

                # ================= LAYER 2 (partials) =================
                pool_ps = psPool.tile([P, P], F32)

                def post_rs(k):
                    ka, kb = cfg.chunks[k]
                    for sc in range(ka, kb):
                        a2 = ep.tile([P, 512], F16, name="a2")
                        nc.sync.dma_start(
                            out=a2[:, :],
                            in_=agg2d[k][(sc - ka) * P:(sc - ka + 1) * P, :])
                        pre = psH.tile([P, 512], F32, name="pre", tag="h")
                        for sub in range(4):
                            col = sc * 4 + sub
                            sd = s_build(sid_sb[:, 0:1], snorm_sb[:, col:col + 1],
                                         name="sd2")
                            nc.tensor.matmul(
                                out=pre[:, sub * P:(sub + 1) * P],
                                lhsT=h_n_all[:, sc * 4 + sub, :], rhs=sd[:],
                                start=(sub == 0), stop=False)
                        nc.tensor.matmul(
                            out=pre[:, 0:512], lhsT=ident16[:],
                            rhs=a2[:, :], start=False, stop=True)
                        fullagg = ep.tile([P, 512], F16, name="fullagg")
                        nc.scalar.activation(out=fullagg[:], in_=pre[:],
                                             func=AF.Copy, bias=0.0, scale=1.0)
                        hps = psH.tile([P, 512], F32, name="hps2", tag="h")
                        nc.tensor.matmul(out=hps[:], lhsT=wsb["W2"][:],
                                         rhs=fullagg[:], start=True, stop=True)
                        h2 = ep.tile([P, 512], F32, name="h2")
                        nc.scalar.activation(out=h2[:], in_=hps[:], func=AF.Relu,
                                             bias=wsb["b2"][:, 0:1], scale=1.0)
                        for sub in range(4):
                            col = sc * 4 + sub
                            tp32 = psT.tile([P, P], F32, name="tp32", tag="tp")
                            nc.tensor.transpose(
                                tp32[:], h2[:, sub * P:(sub + 1) * P], ident32[:])
                            hn32 = sp.tile([P, P], F32, name="hn32")
                            nc.vector.tensor_copy(out=hn32[:], in_=tp32[:])
                            gsel = sp.tile([P, P], F32, name="gsel")
                            nc.vector.tensor_scalar(
                                out=gsel[:], in0=iota32_sb[:],
                                scalar1=gid_sb[:, col:col + 1],
                                scalar2=inv_sb[:, col:col + 1],
                                op0=OP.is_equal, op1=OP.mult,
                            )
                            nc.tensor.matmul(
                                out=pool_ps[:], lhsT=hn32[:], rhs=gsel[:],
                                start=(sc == 0 and sub == 0),
                                stop=(sc == cfg.nsc - 1 and sub == 3))

                for ck, (a, b) in enumerate(cfg.chunks):
                    for sc in range(a, b):
                        T_sc = int(cfg.T2[sc])
                        off = int(cfg.off2[sc])
                        g = None
                        if "noedges" not in ablate:
                            g = gbufp.tile([P, T_sc, P], F16, name="g2")
                            if ck == 0 and sc == a and token16 is not None:
                                # WAW dep: the first gather overwrites this
                                # cell, so it (and, via Pool queue order, all
                                # later gathers) waits for the pair barrier.
                                nc.vector.tensor_copy(out=g[0:1, 0, 0:1],
                                                      in_=token16[:])
                            if "nogather" not in ablate:
                                for c0 in range(0, T_sc, GCALL):
                                    c1 = min(c0 + GCALL, T_sc)
                                    nc.gpsimd.dma_gather(
                                        out_ap=g[:, c0:c1, :],
                                        in_ap=h1_pair[0:2 * block, :],
                                        idxs_ap=idx2t[:, (off + c0) * 8:(off + c1) * 8],
                                        num_idxs=(c1 - c0) * P,
                                        num_idxs_reg=(c1 - c0) * P,
                                        elem_size=P,
                                    )
                            else:
                                nc.vector.tensor_copy(out=g[:, 0, :],
                                                      in_=iota16_sb[:])
                        for j in range(4):
                            agg = psA.tile([P, 512], F32, name="agg")
                            if "noedges" in ablate:
                                nc.tensor.matmul(out=agg[:, 0:512],
                                                 lhsT=ident16[:],
                                                 rhs=xblock_sb[:, 0:4, :].rearrange(
                                                     "p a b -> p (a b)"),
                                                 start=True, stop=True)
                            else:
                                first = True
                                for sub in range(4):
                                    tcnt = int(cfg.t2[sc, j, sub])
                                    t0 = int(cfg.toff2[sc, j, sub])
                                    for t in range(t0, t0 + tcnt):
                                        if "noS" not in ablate:
                                            s_ap = s_build(
                                                slot2t[:, off + t:off + t + 1],
                                                norm2t[:, off + t:off + t + 1])[:]
                                        else:
                                            s_ap = iota16_sb[:]
                                        nc.tensor.matmul(
                                            out=agg[:, sub * P:(sub + 1) * P],
                                            lhsT=g[:, t if "nogather" not in ablate
                                                   else 0, :],
                                            rhs=s_ap, start=first,
                                            stop=(sub == 3 and t == t0 + tcnt - 1))
                                        first = False
                            aggs2 = ep.tile([P, 512], F16, name="aggs2")
                            nc.scalar.activation(out=aggs2[:], in_=agg[:],
                                                 func=AF.Copy, bias=0.0, scale=1.0)
                            nc.sync.dma_start(
                                out=partial[ck][j, (sc - a) * P:(sc - a + 1) * P, :],
                                in_=aggs2[:])
                    # ---- chunk ReduceScatter (delayed by one chunk so the
                    # CC's input wait does not block next-chunk gathers at the
                    # head of the Pool queue) ----
                    def issue_rs(k):
                        if "nocc" in ablate:
                            nc.sync.dma_start(out=agg2d[k][:, :],
                                              in_=partial[k][0, :, :])
                        else:
                            cc_issue(
                                "ReduceScatter", OP.add,
                                replica_groups=[[0, 2, 4, 6], [1, 3, 5, 7]],
                                ins=[partial[k][:, :, :]], outs=[agg2d[k][:, :]],
                            )
                    if RSDELAY:
                        if ck > 0:
                            issue_rs(ck - 1)
                            post_rs(ck - 1)
                        if ck == len(cfg.chunks) - 1:
                            issue_rs(ck)
                            post_rs(ck)
                    else:
                        issue_rs(ck)
                        post_rs(ck)

                # ---- pooled AllReduce + head ----
                pool_sb = ep.tile([P, P], F32, name="pool_sb")
                nc.vector.tensor_copy(out=pool_sb[:], in_=pool_ps[:])
                nc.sync.dma_start(out=pool_loc[:, :], in_=pool_sb[:])
                if "nocc" in ablate:
                    nc.sync.dma_start(out=pool_glob[:, :], in_=pool_loc[:, :])
                else:
                    cc_issue(
                        "AllReduce", OP.add,
                        replica_groups=[list(range(NC))],
                        ins=[pool_loc[:, :]], outs=[pool_glob[:, :]],
                    )
                pooled = ep.tile([P, P], F32, name="pooled")
                nc.sync.dma_start(out=pooled[:], in_=pool_glob[:, :])

                O2 = P // 2
                sm = constp
                zps = psT.tile([O2, P], F32, name="zps", tag="tp")
                nc.tensor.matmul(out=zps[:], lhsT=wsb["fcW1"][:], rhs=pooled[:],
                                 start=True, stop=True)
                z = ep.tile([O2, P], F32, name="z")
                nc.scalar.activation(out=z[:], in_=zps[:], func=AF.Relu,
                                     bias=wsb["fcb1"][:, 0:1], scale=1.0)
                mu = sm.tile([O2, 1], F32, name="mu")
                nc.vector.tensor_reduce(out=mu[:], in_=z[:],
                                        axis=mybir.AxisListType.X, op=OP.add)
                sq = sm.tile([O2, P], F32, name="sq")
                nc.vector.tensor_tensor(out=sq[:], in0=z[:], in1=z[:], op=OP.mult)
                s2m = sm.tile([O2, 1], F32, name="s2m")
                nc.vector.tensor_reduce(out=s2m[:], in_=sq[:],
                                        axis=mybir.AxisListType.X, op=OP.add)
                mu_m = sm.tile([O2, 1], F32, name="mu_m")
                nc.vector.tensor_scalar_mul(mu_m[:], mu[:], 1.0 / NUM_GRAPHS)
                ex2 = sm.tile([O2, 1], F32, name="ex2")
                nc.vector.tensor_scalar_mul(ex2[:], s2m[:], 1.0 / NUM_GRAPHS)
                musq = sm.tile([O2, 1], F32, name="musq")
                nc.vector.tensor_tensor(out=musq[:], in0=mu_m[:], in1=mu_m[:],
                                        op=OP.mult)
                var = sm.tile([O2, 1], F32, name="var")
                nc.vector.tensor_tensor(out=var[:], in0=ex2[:], in1=musq[:],
                                        op=OP.subtract)
                varep = sm.tile([O2, 1], F32, name="varep")
                nc.vector.tensor_scalar_add(varep[:], var[:], BN_EPS)
                sd_ = sm.tile([O2, 1], F32, name="sd_")
                nc.scalar.activation(out=sd_[:], in_=varep[:], func=AF.Sqrt,
                                     bias=0.0, scale=1.0)
                rstd = sm.tile([O2, 1], F32, name="rstd")
                nc.vector.reciprocal(out=rstd[:], in_=sd_[:])
                seff = sm.tile([O2, 1], F32, name="seff")
                nc.vector.tensor_tensor(out=seff[:], in0=rstd[:],
                                        in1=wsb["gamma"][:], op=OP.mult)
                tmp = sm.tile([O2, 1], F32, name="tmp")
                nc.vector.tensor_tensor(out=tmp[:], in0=mu_m[:], in1=seff[:],
                                        op=OP.mult)
                beff = sm.tile([O2, 1], F32, name="beff")
                nc.vector.tensor_tensor(out=beff[:], in0=wsb["beta"][:],
                                        in1=tmp[:], op=OP.subtract)
                zaug = sm.tile([O2 + 1, P], F32, name="zaug")
                nc.vector.tensor_scalar(out=zaug[0:O2, :], in0=z[:],
                                        scalar1=seff[:, 0:1], scalar2=beff[:, 0:1],
                                        op0=OP.mult, op1=OP.add)
                nc.gpsimd.memset(zaug[O2:O2 + 1, :], 1.0)
                fin_ps = psT.tile([P, 1], F32, name="fin_ps", tag="tp")
                nc.tensor.matmul(out=fin_ps[:], lhsT=zaug[:, :],
                                 rhs=wsb["fcW3a"][:, :], start=True, stop=True)
                fin_sb = sm.tile([P, 1], F32, name="fin_sb")
                nc.vector.tensor_copy(out=fin_sb[:], in_=fin_ps[:])
                nc.sync.dma_start(out=out_d[:, :], in_=fin_sb[:])

    nc.compile()
    return nc


def _make_in_maps(cfg, shared, percore, weights):
    in_maps = []
    for c in range(NC):
        m = dict(shared)
        for k in ("idx1", "slot1", "norm1", "idx2", "slot2", "norm2",
                  "xblock", "snorm", "gid", "inv"):
            m[k] = percore[k][c]
        m.update(weights)
        in_maps.append(m)
    return in_maps


def _weights_arrays(W1, b1, W2, b2, fcW1, fcb1, gamma, beta, fcW3, fcb3):
    f = np.float32
    return {
        "W1": np.ascontiguousarray(np.asarray(W1, f).astype(np.float16)),
        "b1": np.ascontiguousarray(np.asarray(b1, f).reshape(-1, 1)),
        "W2": np.ascontiguousarray(np.asarray(W2, f).astype(np.float16)),
        "b2": np.ascontiguousarray(np.asarray(b2, f).reshape(-1, 1)),
        "fcW1": np.ascontiguousarray(fcW1, f),
        "fcb1": np.ascontiguousarray(np.asarray(fcb1, f).reshape(-1, 1)),
        "gamma": np.ascontiguousarray(np.asarray(gamma, f).reshape(-1, 1)),
        "beta": np.ascontiguousarray(np.asarray(beta, f).reshape(-1, 1)),
        "fcW3a": np.ascontiguousarray(
            np.concatenate([np.asarray(fcW3, f).reshape(-1, 1),
                            np.asarray(fcb3, f).reshape(1, 1)], axis=0)),
    }


def _pjrt_bench(nc, in_maps, n_cores, iters=20):
    """Keeps inputs device-resident; times steady-state executions."""
    import time

    import jax
    from jax.experimental.shard_map import shard_map
    from jax.sharding import Mesh, NamedSharding, PartitionSpec

    from concourse import bass2jax

    bass2jax.install_neuronx_cc_hook()
    partition_name = nc.partition_id_tensor.name if nc.partition_id_tensor else None
    in_names, out_names, out_avals, zero_outs = [], [], [], []
    for alloc in nc.m.functions[0].allocations:
        if not isinstance(alloc, mybir.MemoryLocationSet):
            continue
        name = alloc.memorylocations[0].name
        if alloc.kind == "ExternalInput":
            if name != partition_name:
                in_names.append(name)
        elif alloc.kind == "ExternalOutput":
            out_names.append(name)
            shape = tuple(alloc.tensor_shape)
            dtype = mybir.dt.np(alloc.dtype)
            out_avals.append(jax.core.ShapedArray(shape, dtype))
            zero_outs.append(np.zeros(shape, dtype))
    n_params = len(in_names)
    n_outs = len(out_avals)
    in_names_all = list(in_names) + out_names
    if partition_name is not None:
        in_names_all.append(partition_name)

    def _body(*args):
        operands = list(args)
        if partition_name is not None:
            operands.append(bass2jax.partition_id_tensor())
        outs = bass2jax._bass_exec_p.bind(
            *operands,
            out_avals=tuple(out_avals),
            in_names=tuple(in_names_all),
            out_names=tuple(out_names),
            lowering_input_output_aliases=(),
            sim_require_finite=True,
            sim_require_nnan=True,
            nc=nc,
        )
        return tuple(outs)

    devices = jax.devices()[:n_cores]
    mesh = Mesh(np.asarray(devices), ("core",))
    donate = tuple(range(n_params, n_params + n_outs))
    sharded = jax.jit(
        shard_map(_body, mesh=mesh,
                  in_specs=(PartitionSpec("core"),) * (n_params + n_outs),
                  out_specs=(PartitionSpec("core"),) * n_outs, check_rep=False),
        donate_argnums=donate, keep_unused=True,
    )
    spec = NamedSharding(mesh, PartitionSpec("core"))
    concat_in = [
        jax.device_put(
            np.concatenate([np.asarray(in_maps[c][nm]) for c in range(n_cores)],
                           axis=0), spec)
        for nm in in_names
    ]
    for a in concat_in:
        a.block_until_ready()

    def zeros():
        return [np.zeros((n_cores * z.shape[0], *z.shape[1:]), z.dtype)
                for z in zero_outs]

    out_arrs = sharded(*concat_in, *zeros())
    jax.block_until_ready(out_arrs)
    results = [
        {nm: np.asarray(out_arrs[i]).reshape(n_cores, *out_avals[i].shape)[c]
         for i, nm in enumerate(out_names)}
        for c in range(n_cores)
    ]
    import time as _t
    t0 = _t.perf_counter()
    last = None
    for _ in range(iters):
        last = sharded(*concat_in, *zeros())
    jax.block_until_ready(last)
    per_iter_ns = (_t.perf_counter() - t0) / iters * 1e9
    return results, per_iter_ns


def run(inputs, trace=False, nc_cores=8):
    cfg, shared, percore = _prep(inputs["x"], inputs["edge_index"],
                                 inputs["batch"], nc_cores=nc_cores)
    weights = _weights_arrays(
        inputs["W1"], inputs["b1"], inputs["W2"], inputs["b2"],
        inputs["fcW1"], inputs["fcb1"], inputs["gamma"], inputs["beta"],
        inputs["fcW3"], inputs["fcb3"])
    nc = _build(cfg)
    in_maps = _make_in_maps(cfg, shared, percore, weights)
    if trace:
        results, per_iter_ns = _pjrt_bench(nc, in_maps, cfg.nc)
        out = np.asarray(results[0]["out"], np.float32).reshape(NUM_GRAPHS, 1)
        return out, per_iter_ns
    res = run_bass_kernel_spmd(nc, in_maps, list(range(cfg.nc)), trace=False)
    out = np.asarray(res.results[0]["out"], np.float32).reshape(NUM_GRAPHS, 1)
    return out, res.exec_time_ns


def kernel(**inputs) -> np.ndarray:
    out, _ = run(inputs, trace=False)
    return out
